# revision 10
# baseline (speedup 1.0000x reference)
"""Trainium2 Bass kernel for nn_CrossXMFusion (dense_transformer).

Computation per batch n (C=1024 channels, T=2048 time):
  S    = X @ M^T / T                  (attention logits, contraction over t)
  A    = softmax(S, axis=-1)
  Vx   = A^T @ X + X                  (cross_x)
  Vm   = A @ M + M                    (cross_m)
  h(V) = gelu(V^T @ W1^T + b1) @ W2^T + b2   (channel-FFN, t-parallel)
  out_x = h(Vx)^T + X ; out_m = h(Vm)^T + M

Sharding: data-parallel over batch n across 8 NeuronCores (2 batches/core),
FF weights replicated, no cross-device comms. Matmuls run in bf16 with fp32
PSUM accumulation; softmax + final residual adds in fp32.

Data-feed strategy (v2): SWDGE casting DMAs measured ~55 GB/s on HW, so all
HBM loads are plain fp32 on the two HWDGE queues (scalar=staging loads,
sync=transposes+residuals) and fp32->bf16 casts run on Vector/Scalar engines.
X^T / M^T / A^T are produced by DMA xbar transposes (natural 3D mapping,
keeps TensorE free); X/M stream through small staged tiles twice (transpose
feed + cross-stage rhs) to fit SBUF; output stores go on the (otherwise idle)
SWDGE queue.
"""

import sys

sys.path.insert(0, "/opt/trn_rl_repo")

import numpy as np
import ml_dtypes

NCORES = 8
NFULL = 16  # full batch
NB = NFULL // NCORES  # batches per core
C, T, P = 1024, 2048, 128
CO = C // P  # 8 channel tiles
TO = T // P  # 16 time tiles
TB = 512  # matmul moving free-dim block
NT = T // TB  # 4 t-blocks
DB = 512  # stage-A d block
ND = C // DB  # 2

_CACHE = {}


def _build(nb=NB, act_name="Gelu"):
    import concourse.mybir as mybir
    import concourse.tile as tile
    from concourse import bacc

    dt = mybir.dt
    AF = mybir.ActivationFunctionType
    AF_ACT = getattr(AF, act_name)
    bf16 = dt.bfloat16
    f32 = dt.float32

    nc = bacc.Bacc("TRN2", target_bir_lowering=False, debug=False, num_devices=NCORES)

    fx = nc.dram_tensor("feature_x", [nb, C, T], f32, kind="ExternalInput")
    fm = nc.dram_tensor("feature_m", [nb, C, T], f32, kind="ExternalInput")
    w1t = nc.dram_tensor("w1t", [C, C], bf16, kind="ExternalInput")
    w2t = nc.dram_tensor("w2t", [C, C], bf16, kind="ExternalInput")
    b1 = nc.dram_tensor("b1", [C], f32, kind="ExternalInput")
    b2 = nc.dram_tensor("b2", [C], f32, kind="ExternalInput")
    out_x = nc.dram_tensor("out_x", [nb, C, T], f32, kind="ExternalOutput")
    out_m = nc.dram_tensor("out_m", [nb, C, T], f32, kind="ExternalOutput")

    with tile.TileContext(nc) as tc:
        with (
            tc.tile_pool(name="const", bufs=1) as constp,
            tc.tile_pool(name="stgp", bufs=6) as stgp,
            tc.tile_pool(name="xcp", bufs=13) as xcp,
            tc.tile_pool(name="shp", bufs=2) as shp,
            tc.tile_pool(name="amp", bufs=1) as amp,
            tc.tile_pool(name="h1p", bufs=1) as h1p,
            tc.tile_pool(name="residp", bufs=3) as residp,
            tc.tile_pool(name="outstp", bufs=3) as outstp,
            tc.tile_pool(name="statp", bufs=2) as statp,
            tc.tile_pool(name="psp", bufs=8, space="PSUM") as psp,
        ):
            # ---- persistent weights/biases ----
            w1t_sb = constp.tile([P, CO, C], bf16, tag="w1", name="w1t_sb")
            nc.scalar.dma_start(w1t_sb[:], w1t.rearrange("(co p) j -> p co j", p=P))
            w2t_sb = constp.tile([P, CO, C], bf16, tag="w2", name="w2t_sb")
            nc.scalar.dma_start(w2t_sb[:], w2t.rearrange("(jo p) i -> p jo i", p=P))
            b1_sb = constp.tile([P, CO], f32, tag="b1", name="b1_sb")
            nc.scalar.dma_start(b1_sb[:], b1.rearrange("(jo p) -> p jo", p=P))
            b2_sb = constp.tile([P, CO], f32, tag="b2", name="b2_sb")
            nc.scalar.dma_start(b2_sb[:], b2.rearrange("(io p) -> p io", p=P))

            def stage_cast(src, eng, name):
                """HWDGE fp32 load of [P, C] slice + on-chip cast to bf16."""
                stg = stgp.tile([P, C], f32, tag="stg", name=f"stg_{name}")
                nc.scalar.dma_start(stg, src)
                out = xcp.tile([P, C], bf16, tag="xc", name=f"bf_{name}")
                if eng == "v":
                    nc.vector.tensor_copy(out, stg)
                else:
                    nc.scalar.copy(out, stg)
                return out

            for n in range(nb):
                # ---- load + cast + transpose (M rows 0-3, X all, M rows
                # 4-7 — the order stage A consumes them) ----
                mt = shp.tile([P, TO, C], bf16, tag="sh", name=f"mt{n}")
                xt = shp.tile([P, TO, C], bf16, tag="sh", name=f"xt{n}")

                def row_T(dst, src_dram, co, h, name):
                    rb = stage_cast(
                        src_dram[n, co * P : (co + 1) * P, h * C : (h + 1) * C],
                        "v",
                        name,
                    )
                    nc.sync.dma_start(
                        dst[:, h * CO : (h + 1) * CO, co * P : (co + 1) * P],
                        rb,
                        transpose=True,
                    )

                for co in range(4):
                    for h in range(2):
                        row_T(mt, fm, co, h, f"m{n}_{co}_{h}")
                for co in range(CO):
                    for h in range(2):
                        row_T(xt, fx, co, h, f"x{n}_{co}_{h}")
                for co in range(4, CO):
                    for h in range(2):
                        row_T(mt, fm, co, h, f"m{n}_{co}_{h}")

                # ---- stage A: S = X M^T / T, A = softmax rows ----
                a = amp.tile([P, CO, C], bf16, tag="a", name=f"a{n}")
                rs2 = statp.tile([P, ND, CO], f32, tag="rs2", name=f"rs2_{n}")
                rs = statp.tile([P, CO], f32, tag="rs", name=f"rs_{n}")
                rinv = statp.tile([P, CO], f32, tag="rinv", name=f"rinv_{n}")
                for db in range(ND):
                    for co in range(CO):
                        ps = psp.tile(
                            [P, DB], f32, tag="mm", name=f"psA{n}_{co}_{db}"
                        )
                        for to in range(TO):
                            nc.tensor.matmul(
                                ps,
                                xt[:, to, co * P : (co + 1) * P],
                                mt[:, to, db * DB : (db + 1) * DB],
                                start=(to == 0),
                                stop=(to == TO - 1),
                            )
                        # A_raw = exp(S/T); row-sums accumulate for softmax.
                        nc.scalar.activation(
                            a[:, co, db * DB : (db + 1) * DB],
                            ps,
                            AF.Exp,
                            scale=1.0 / T,
                            accum_out=rs2[:, db, co : co + 1],
                        )
                nc.vector.tensor_add(rs, rs2[:, 0, :], rs2[:, 1, :])
                nc.vector.reciprocal(rinv, rs)
                for co in range(CO):
                    nc.vector.tensor_scalar_mul(
                        a[:, co, :], a[:, co, :], rinv[:, co : co + 1]
                    )
                # A^T via xbar (exact: natural mapping)
                at = amp.tile([P, CO, C], bf16, tag="at", name=f"at{n}")
                for co in range(CO):
                    nc.sync.dma_start(
                        at[:, :, co * P : (co + 1) * P], a[:, co, :], transpose=True
                    )

                # ---- cross_x = A^T X + X ----
                vx = shp.tile([P, CO, T], bf16, tag="sh", name=f"vx{n}")
                for h in range(2):
                    xc = [
                        stage_cast(
                            fx[n, co * P : (co + 1) * P, h * C : (h + 1) * C],
                            "s",
                            f"xc{n}_{h}_{co}",
                        )
                        for co in range(CO)
                    ]
                    for lb in range(2):
                        tb = h * 2 + lb
                        for do in range(CO):
                            ps = psp.tile(
                                [P, TB], f32, tag="mm", name=f"psX{n}_{tb}_{do}"
                            )
                            for co in range(CO):
                                nc.tensor.matmul(
                                    ps,
                                    a[:, co, do * P : (do + 1) * P],
                                    xc[co][:, lb * TB : (lb + 1) * TB],
                                    start=(co == 0),
                                    stop=(co == CO - 1),
                                )
                            nc.vector.tensor_add(
                                vx[:, do, tb * TB : (tb + 1) * TB],
                                ps,
                                xc[do][:, lb * TB : (lb + 1) * TB],
                            )

                # ---- cross_m = A M + M ----
                vm = shp.tile([P, CO, T], bf16, tag="sh", name=f"vm{n}")
                for h in range(2):
                    mc = [
                        stage_cast(
                            fm[n, do * P : (do + 1) * P, h * C : (h + 1) * C],
                            "s",
                            f"mc{n}_{h}_{do}",
                        )
                        for do in range(CO)
                    ]
                    for lb in range(2):
                        tb = h * 2 + lb
                        for co in range(CO):
                            ps = psp.tile(
                                [P, TB], f32, tag="mm", name=f"psM{n}_{tb}_{co}"
                            )
                            for do in range(CO):
                                nc.tensor.matmul(
                                    ps,
                                    at[:, do, co * P : (co + 1) * P],
                                    mc[do][:, lb * TB : (lb + 1) * TB],
                                    start=(do == 0),
                                    stop=(do == CO - 1),
                                )
                            nc.vector.tensor_add(
                                vm[:, co, tb * TB : (tb + 1) * TB],
                                ps,
                                mc[co][:, lb * TB : (lb + 1) * TB],
                            )

                # ---- FFN on both sequences + final residual ----
                for seq, (v, rsrc, odst) in enumerate(
                    [(vx, fx, out_x), (vm, fm, out_m)]
                ):
                    for tb in range(NT):
                        h1 = h1p.tile(
                            [P, CO, TB], bf16, tag="h1", name=f"h1_{n}_{seq}_{tb}"
                        )
                        for jo in range(CO):
                            ps = psp.tile(
                                [P, TB], f32, tag="mm", name=f"ps1_{n}_{seq}_{tb}_{jo}"
                            )
                            for co in range(CO):
                                nc.tensor.matmul(
                                    ps,
                                    w1t_sb[:, co, jo * P : (jo + 1) * P],
                                    v[:, co, tb * TB : (tb + 1) * TB],
                                    start=(co == 0),
                                    stop=(co == CO - 1),
                                )
                            nc.scalar.activation(
                                h1[:, jo, :],
                                ps,
                                AF_ACT,
                                bias=b1_sb[:, jo : jo + 1],
                                scale=1.0,
                            )
                        for io in range(CO):
                            ps = psp.tile(
                                [P, TB], f32, tag="mm", name=f"ps2_{n}_{seq}_{tb}_{io}"
                            )
                            for jo in range(CO):
                                nc.tensor.matmul(
                                    ps,
                                    w2t_sb[:, jo, io * P : (io + 1) * P],
                                    h1[:, jo, :],
                                    start=(jo == 0),
                                    stop=(jo == CO - 1),
                                )
                            st = outstp.tile(
                                [P, TB], f32, tag="outst", name=f"st_{n}_{seq}_{tb}_{io}"
                            )
                            nc.scalar.activation(
                                st, ps, AF.Identity, bias=b2_sb[:, io : io + 1]
                            )
                            rt = residp.tile(
                                [P, TB], f32, tag="resid", name=f"rt_{n}_{seq}_{tb}_{io}"
                            )
                            nc.sync.dma_start(
                                rt,
                                rsrc[
                                    n,
                                    io * P : (io + 1) * P,
                                    tb * TB : (tb + 1) * TB,
                                ],
                            )
                            nc.vector.tensor_add(st, st, rt)
                            nc.gpsimd.dma_start(
                                odst[
                                    n,
                                    io * P : (io + 1) * P,
                                    tb * TB : (tb + 1) * TB,
                                ],
                                st,
                            )

    nc.compile()
    return nc


def get_nc(nb=NB):
    if nb not in _CACHE:
        _CACHE[nb] = _build(nb)
    return _CACHE[nb]


def make_in_maps(feature_x, feature_m, W1, b1, W2, b2):
    """Slice full inputs into 8 per-core input maps (host-side prep)."""
    fx = np.ascontiguousarray(np.asarray(feature_x, dtype=np.float32))
    fm = np.ascontiguousarray(np.asarray(feature_m, dtype=np.float32))
    w1t = np.ascontiguousarray(np.asarray(W1, dtype=np.float32).T).astype(
        ml_dtypes.bfloat16
    )
    w2t = np.ascontiguousarray(np.asarray(W2, dtype=np.float32).T).astype(
        ml_dtypes.bfloat16
    )
    b1 = np.ascontiguousarray(np.asarray(b1, dtype=np.float32))
    b2 = np.ascontiguousarray(np.asarray(b2, dtype=np.float32))
    in_maps = []
    for k in range(NCORES):
        in_maps.append(
            {
                "feature_x": fx[k * NB : (k + 1) * NB],
                "feature_m": fm[k * NB : (k + 1) * NB],
                "w1t": w1t,
                "w2t": w2t,
                "b1": b1,
                "b2": b2,
            }
        )
    return in_maps


def run_device(in_maps, trace=False, trace_kwargs=None):
    from concourse.bass_utils import run_bass_kernel_spmd

    nc = get_nc(NB)
    return run_bass_kernel_spmd(
        nc,
        in_maps,
        core_ids=list(range(NCORES)),
        trace=trace,
        **(trace_kwargs or {}),
    )


def kernel(feature_x, feature_m, W1, b1, W2, b2):
    in_maps = make_in_maps(feature_x, feature_m, W1, b1, W2, b2)
    res = run_device(in_maps, trace=False)
    out_x = np.concatenate([r["out_x"] for r in res.results], axis=0)
    out_m = np.concatenate([r["out_m"] for r in res.results], axis=0)
    return (out_x.astype(np.float32), out_m.astype(np.float32))


# revision 17
# speedup vs baseline: 1.2693x; 1.2693x over previous
"""Trainium2 Bass kernel for nn_CrossXMFusion (dense_transformer).

Computation per batch n (C=1024 channels, T=2048 time):
  S    = X @ M^T / T                  (attention logits, contraction over t)
  A    = softmax(S, axis=-1)
  Vx   = A^T @ X + X                  (cross_x)
  Vm   = A @ M + M                    (cross_m)
  h(V) = gelu(V^T @ W1^T + b1) @ W2^T + b2   (channel-FFN, t-parallel)
  out_x = h(Vx)^T + X ; out_m = h(Vm)^T + M

Sharding: data-parallel over batch n across 8 NeuronCores (2 batches/core),
FF weights replicated, no cross-device comms.

Design notes (v4), driven by HW traces:
 - Matmuls in bf16 with fp32 PSUM accumulation; softmax in fp32.
 - Per-NC HBM one-way bandwidth measured ~200 GB/s and DMAs within one
   engine-queue complete serially, so input features are pre-cast to bf16 on
   the host (same RNE rounding the device DMA cast would apply) — halves load
   bytes and avoids the (measured ~55 GB/s) SWDGE casting-DMA path entirely.
 - X rows load on the scalar HWDGE ring, M rows on the SWDGE ring (parallel),
   transposes + small loads on the sync HWDGE ring.
 - X^T / M^T / A^T via DMA xbar transposes (exact natural 3D mapping), PE left
   entirely to matmuls.
 - Phase order per batch: stage A -> cross_x -> FFN(x) -> cross_m -> FFN(m);
   X and M bf16 rows stay resident for transpose feed, cross rhs, and
   residuals (no reloads); slot rotation frees X rows to the next batch after
   FFN(x) and M rows after cross_m.
"""

import sys

sys.path.insert(0, "/opt/trn_rl_repo")

import numpy as np
import ml_dtypes

NCORES = 8
NFULL = 16  # full batch
NB = NFULL // NCORES  # batches per core
C, T, P = 1024, 2048, 128
CO = C // P  # 8 channel tiles
TO = T // P  # 16 time tiles
TB = 512  # matmul moving free-dim block
NT = T // TB  # 4 t-blocks
DB = 512  # stage-A d block
ND = C // DB  # 2

_CACHE = {}


def _build(nb=NB, act_name="Gelu"):
    import concourse.mybir as mybir
    import concourse.tile as tile
    from concourse import bacc

    dt = mybir.dt
    AF = mybir.ActivationFunctionType
    AF_ACT = getattr(AF, act_name)
    bf16 = dt.bfloat16
    f32 = dt.float32

    nc = bacc.Bacc("TRN2", target_bir_lowering=False, debug=False, num_devices=NCORES)

    fxb = nc.dram_tensor("fxb", [nb, C, T], bf16, kind="ExternalInput")
    fmb = nc.dram_tensor("fmb", [nb, C, T], bf16, kind="ExternalInput")
    w1t = nc.dram_tensor("w1t", [C, C], bf16, kind="ExternalInput")
    w2t = nc.dram_tensor("w2t", [C, C], bf16, kind="ExternalInput")
    b1 = nc.dram_tensor("b1", [C], f32, kind="ExternalInput")
    b2 = nc.dram_tensor("b2", [C], f32, kind="ExternalInput")
    out_x = nc.dram_tensor("out_x", [nb, C, T], f32, kind="ExternalOutput")
    out_m = nc.dram_tensor("out_m", [nb, C, T], f32, kind="ExternalOutput")

    with tile.TileContext(nc) as tc:
        with (
            tc.tile_pool(name="const", bufs=1) as constp,
            # bf16 feature rows [P, T]: transpose feed + cross rhs + residuals
            tc.tile_pool(name="xcp", bufs=16) as xcp,
            tc.tile_pool(name="shp", bufs=2) as shp,
            tc.tile_pool(name="amp", bufs=1) as amp,
            tc.tile_pool(name="h1p", bufs=1) as h1p,
            tc.tile_pool(name="outstp", bufs=2) as outstp,
            tc.tile_pool(name="residp", bufs=3) as residp,
            tc.tile_pool(name="statp", bufs=1) as statp,
            tc.tile_pool(name="psp", bufs=8, space="PSUM") as psp,
        ):
            # ---- persistent weights/biases ----
            w1t_sb = constp.tile([P, CO, C], bf16, tag="w1", name="w1t_sb")
            nc.scalar.dma_start(w1t_sb[:], w1t.rearrange("(co p) j -> p co j", p=P))
            w2t_sb = constp.tile([P, CO, C], bf16, tag="w2", name="w2t_sb")
            nc.scalar.dma_start(w2t_sb[:], w2t.rearrange("(jo p) i -> p jo i", p=P))
            b1_sb = constp.tile([P, CO], f32, tag="b1", name="b1_sb")
            nc.scalar.dma_start(b1_sb[:], b1.rearrange("(jo p) -> p jo", p=P))
            b2_sb = constp.tile([P, CO], f32, tag="b2", name="b2_sb")
            nc.scalar.dma_start(b2_sb[:], b2.rearrange("(io p) -> p io", p=P))

            for n in range(nb):
                # ---- bf16 row loads on two parallel rings + xbar transposes.
                # Alloc order (M rows then X rows) sets the slot rotation: the
                # next batch's M rows reuse these M slots (free after cross_m)
                # and its X rows reuse the X slots (free after FFN(x)).
                mt = shp.tile([P, TO, C], bf16, tag="sh", name=f"mt{n}")
                xt = shp.tile([P, TO, C], bf16, tag="sh", name=f"xt{n}")
                mbf = [
                    xcp.tile([P, T], bf16, tag="xc", name=f"mbf{n}_{c}")
                    for c in range(CO)
                ]
                xrows = [
                    xcp.tile([P, T], bf16, tag="xc", name=f"xbf{n}_{c}")
                    for c in range(CO)
                ]
                for co in range(CO):
                    nc.gpsimd.dma_start(mbf[co], fmb[n, co * P : (co + 1) * P, :])
                    nc.scalar.dma_start(xrows[co], fxb[n, co * P : (co + 1) * P, :])

                def row_T(dst, rb, co):
                    nc.sync.dma_start(
                        dst[:, :, co * P : (co + 1) * P], rb, transpose=True
                    )

                # stage A consumes M^T cols 0-511 (M rows 0-3) + all of X^T
                # first, then M rows 4-7
                for co in range(4):
                    row_T(mt, mbf[co], co)
                    row_T(xt, xrows[co], co)
                for co in range(4, CO):
                    row_T(xt, xrows[co], co)
                for co in range(4, CO):
                    row_T(mt, mbf[co], co)

                # ---- stage A: S = X M^T / T, A = softmax rows ----
                a = amp.tile([P, CO, C], bf16, tag="a", name=f"a{n}")
                rs2 = statp.tile([P, ND, CO], f32, tag="rs2", name=f"rs2_{n}")
                for db in range(ND):
                    for co in range(CO):
                        ps = psp.tile(
                            [P, DB], f32, tag="mm", name=f"psA{n}_{co}_{db}"
                        )
                        for to in range(TO):
                            nc.tensor.matmul(
                                ps,
                                xt[:, to, co * P : (co + 1) * P],
                                mt[:, to, db * DB : (db + 1) * DB],
                                start=(to == 0),
                                stop=(to == TO - 1),
                            )
                        # A_raw = exp(S/T); row-sums accumulate for softmax.
                        nc.scalar.activation(
                            a[:, co, db * DB : (db + 1) * DB],
                            ps,
                            AF.Exp,
                            scale=1.0 / T,
                            accum_out=rs2[:, db, co : co + 1],
                        )
                nc.vector.tensor_add(rs2[:, 0, :], rs2[:, 0, :], rs2[:, 1, :])
                rinv = rs2[:, 1, :]
                nc.vector.reciprocal(rinv, rs2[:, 0, :])
                for co in range(CO):
                    nc.vector.tensor_scalar_mul(
                        a[:, co, :], a[:, co, :], rinv[:, co : co + 1]
                    )
                # A^T via xbar (exact: natural mapping)
                at = amp.tile([P, CO, C], bf16, tag="at", name=f"at{n}")
                for co in range(CO):
                    nc.sync.dma_start(
                        at[:, :, co * P : (co + 1) * P], a[:, co, :], transpose=True
                    )

                def cross(v, lhs_t, rows, name):
                    """v[:, o, t] = sum_k lhs_t[:, k, o-tile].T @ rows[k] + rows[o]"""
                    for tb in range(NT):
                        for o in range(CO):
                            ps = psp.tile(
                                [P, TB], f32, tag="mm", name=f"ps{name}_{tb}_{o}"
                            )
                            for k in range(CO):
                                nc.tensor.matmul(
                                    ps,
                                    lhs_t[:, k, o * P : (o + 1) * P],
                                    rows[k][:, tb * TB : (tb + 1) * TB],
                                    start=(k == 0),
                                    stop=(k == CO - 1),
                                )
                            nc.vector.tensor_add(
                                v[:, o, tb * TB : (tb + 1) * TB],
                                ps,
                                rows[o][:, tb * TB : (tb + 1) * TB],
                            )

                def ffn(v, resid_of, odst, seq, store_eng):
                    for tb in range(NT):
                        h1 = h1p.tile(
                            [P, CO, TB], bf16, tag="h1", name=f"h1_{n}_{seq}_{tb}"
                        )
                        for jo in range(CO):
                            ps = psp.tile(
                                [P, TB], f32, tag="mm", name=f"ps1_{n}_{seq}_{tb}_{jo}"
                            )
                            for co in range(CO):
                                nc.tensor.matmul(
                                    ps,
                                    w1t_sb[:, co, jo * P : (jo + 1) * P],
                                    v[:, co, tb * TB : (tb + 1) * TB],
                                    start=(co == 0),
                                    stop=(co == CO - 1),
                                )
                            nc.scalar.activation(
                                h1[:, jo, :],
                                ps,
                                AF_ACT,
                                bias=b1_sb[:, jo : jo + 1],
                                scale=1.0,
                            )
                        for io in range(CO):
                            ps = psp.tile(
                                [P, TB], f32, tag="mm", name=f"ps2_{n}_{seq}_{tb}_{io}"
                            )
                            for jo in range(CO):
                                nc.tensor.matmul(
                                    ps,
                                    w2t_sb[:, jo, io * P : (io + 1) * P],
                                    h1[:, jo, :],
                                    start=(jo == 0),
                                    stop=(jo == CO - 1),
                                )
                            st = outstp.tile(
                                [P, TB],
                                f32,
                                tag="outst",
                                name=f"st_{n}_{seq}_{tb}_{io}",
                            )
                            nc.scalar.activation(
                                st, ps, AF.Identity, bias=b2_sb[:, io : io + 1]
                            )
                            rt = resid_of(io, tb)
                            nc.vector.tensor_add(st, st, rt)
                            getattr(nc, store_eng).dma_start(
                                odst[
                                    n,
                                    io * P : (io + 1) * P,
                                    tb * TB : (tb + 1) * TB,
                                ],
                                st,
                            )

                # ---- cross_x = A^T X + X (X rows resident) ----
                vx = shp.tile([P, CO, T], bf16, tag="sh", name=f"vx{n}")
                cross(vx, a, xrows, f"X{n}")

                # x-sequence FFN residual straight from the live X rows
                ffn(
                    vx,
                    lambda io, tb: xrows[io][:, tb * TB : (tb + 1) * TB],
                    out_x,
                    0,
                    "scalar",
                )

                # ---- cross_m = A M + M (M rows resident) ----
                vm = shp.tile([P, CO, T], bf16, tag="sh", name=f"vm{n}")
                cross(vm, at, mbf, f"M{n}")

                # m-sequence FFN residual from fresh small bf16 loads so the
                # M row slots free at cross_m (keeps next batch's loads early)
                def m_resid(io, tb):
                    rt = residp.tile(
                        [P, TB], bf16, tag="res", name=f"rt_{n}_{io}_{tb}"
                    )
                    nc.sync.dma_start(
                        rt, fmb[n, io * P : (io + 1) * P, tb * TB : (tb + 1) * TB]
                    )
                    return rt

                ffn(vm, m_resid, out_m, 1, "gpsimd")

    nc.compile()
    return nc


def get_nc(nb=NB):
    if nb not in _CACHE:
        _CACHE[nb] = _build(nb)
    return _CACHE[nb]


def make_in_maps(feature_x, feature_m, W1, b1, W2, b2):
    """Host prep: slice per core + pre-cast features/weights to bf16."""
    fxb = np.asarray(feature_x, dtype=np.float32).astype(ml_dtypes.bfloat16)
    fmb = np.asarray(feature_m, dtype=np.float32).astype(ml_dtypes.bfloat16)
    w1t = np.ascontiguousarray(np.asarray(W1, dtype=np.float32).T).astype(
        ml_dtypes.bfloat16
    )
    w2t = np.ascontiguousarray(np.asarray(W2, dtype=np.float32).T).astype(
        ml_dtypes.bfloat16
    )
    b1 = np.ascontiguousarray(np.asarray(b1, dtype=np.float32))
    b2 = np.ascontiguousarray(np.asarray(b2, dtype=np.float32))
    in_maps = []
    for k in range(NCORES):
        in_maps.append(
            {
                "fxb": np.ascontiguousarray(fxb[k * NB : (k + 1) * NB]),
                "fmb": np.ascontiguousarray(fmb[k * NB : (k + 1) * NB]),
                "w1t": w1t,
                "w2t": w2t,
                "b1": b1,
                "b2": b2,
            }
        )
    return in_maps


def run_device(in_maps, trace=False, trace_kwargs=None):
    from concourse.bass_utils import run_bass_kernel_spmd

    nc = get_nc(NB)
    return run_bass_kernel_spmd(
        nc,
        in_maps,
        core_ids=list(range(NCORES)),
        trace=trace,
        **(trace_kwargs or {}),
    )


def kernel(feature_x, feature_m, W1, b1, W2, b2):
    in_maps = make_in_maps(feature_x, feature_m, W1, b1, W2, b2)
    res = run_device(in_maps, trace=False)
    out_x = np.concatenate([r["out_x"] for r in res.results], axis=0)
    out_m = np.concatenate([r["out_m"] for r in res.results], axis=0)
    return (out_x.astype(np.float32), out_m.astype(np.float32))


# revision 18
# speedup vs baseline: 1.2729x; 1.0029x over previous
"""Trainium2 Bass kernel for nn_CrossXMFusion (dense_transformer).

Computation per batch n (C=1024 channels, T=2048 time):
  S    = X @ M^T / T                  (attention logits, contraction over t)
  A    = softmax(S, axis=-1)
  Vx   = A^T @ X + X                  (cross_x)
  Vm   = A @ M + M                    (cross_m)
  h(V) = gelu(V^T @ W1^T + b1) @ W2^T + b2   (channel-FFN, t-parallel)
  out_x = h(Vx)^T + X ; out_m = h(Vm)^T + M

Sharding: data-parallel over batch n across 8 NeuronCores (2 batches/core),
FF weights replicated, no cross-device comms.

Design notes (v4), driven by HW traces:
 - Matmuls in bf16 with fp32 PSUM accumulation; softmax in fp32.
 - Per-NC HBM one-way bandwidth measured ~200 GB/s and DMAs within one
   engine-queue complete serially, so input features are pre-cast to bf16 on
   the host (same RNE rounding the device DMA cast would apply) — halves load
   bytes and avoids the (measured ~55 GB/s) SWDGE casting-DMA path entirely.
 - X rows load on the scalar HWDGE ring, M rows on the SWDGE ring (parallel),
   transposes + small loads on the sync HWDGE ring.
 - X^T / M^T / A^T via DMA xbar transposes (exact natural 3D mapping), PE left
   entirely to matmuls.
 - Phase order per batch: stage A -> cross_x -> FFN(x) -> cross_m -> FFN(m);
   X and M bf16 rows stay resident for transpose feed, cross rhs, and
   residuals (no reloads); slot rotation frees X rows to the next batch after
   FFN(x) and M rows after cross_m.
"""

import sys

sys.path.insert(0, "/opt/trn_rl_repo")

import numpy as np
import ml_dtypes

NCORES = 8
NFULL = 16  # full batch
NB = NFULL // NCORES  # batches per core
C, T, P = 1024, 2048, 128
CO = C // P  # 8 channel tiles
TO = T // P  # 16 time tiles
TB = 512  # matmul moving free-dim block
NT = T // TB  # 4 t-blocks
DB = 512  # stage-A d block
ND = C // DB  # 2

_CACHE = {}


def _build(nb=NB, act_name="Gelu"):
    import concourse.mybir as mybir
    import concourse.tile as tile
    from concourse import bacc

    dt = mybir.dt
    AF = mybir.ActivationFunctionType
    AF_ACT = getattr(AF, act_name)
    bf16 = dt.bfloat16
    f32 = dt.float32

    nc = bacc.Bacc("TRN2", target_bir_lowering=False, debug=False, num_devices=NCORES)

    fxb = nc.dram_tensor("fxb", [nb, C, T], bf16, kind="ExternalInput")
    fmb = nc.dram_tensor("fmb", [nb, C, T], bf16, kind="ExternalInput")
    w1t = nc.dram_tensor("w1t", [C, C], bf16, kind="ExternalInput")
    w2t = nc.dram_tensor("w2t", [C, C], bf16, kind="ExternalInput")
    b1 = nc.dram_tensor("b1", [C], f32, kind="ExternalInput")
    b2 = nc.dram_tensor("b2", [C], f32, kind="ExternalInput")
    out_x = nc.dram_tensor("out_x", [nb, C, T], f32, kind="ExternalOutput")
    out_m = nc.dram_tensor("out_m", [nb, C, T], f32, kind="ExternalOutput")

    with tile.TileContext(nc) as tc:
        with (
            tc.tile_pool(name="const", bufs=1) as constp,
            # bf16 feature rows [P, T]: transpose feed + cross rhs + residuals
            tc.tile_pool(name="xcp", bufs=16) as xcp,
            tc.tile_pool(name="shp", bufs=2) as shp,
            tc.tile_pool(name="amp", bufs=1) as amp,
            tc.tile_pool(name="h1p", bufs=1) as h1p,
            tc.tile_pool(name="outstp", bufs=2) as outstp,
            tc.tile_pool(name="residp", bufs=3) as residp,
            tc.tile_pool(name="statp", bufs=1) as statp,
            tc.tile_pool(name="psp", bufs=8, space="PSUM") as psp,
        ):
            # ---- persistent weights/biases (tiles here; DMAs issued after
            # batch 0's feature rows so they don't block the scalar ring) ----
            w1t_sb = constp.tile([P, CO, C], bf16, tag="w1", name="w1t_sb")
            w2t_sb = constp.tile([P, CO, C], bf16, tag="w2", name="w2t_sb")
            b1_sb = constp.tile([P, CO], f32, tag="b1", name="b1_sb")
            b2_sb = constp.tile([P, CO], f32, tag="b2", name="b2_sb")

            for n in range(nb):
                # ---- bf16 row loads on two parallel rings + xbar transposes.
                # Alloc order (M rows then X rows) sets the slot rotation: the
                # next batch's M rows reuse these M slots (free after cross_m)
                # and its X rows reuse the X slots (free after FFN(x)).
                mt = shp.tile([P, TO, C], bf16, tag="sh", name=f"mt{n}")
                xt = shp.tile([P, TO, C], bf16, tag="sh", name=f"xt{n}")
                mbf = [
                    xcp.tile([P, T], bf16, tag="xc", name=f"mbf{n}_{c}")
                    for c in range(CO)
                ]
                xrows = [
                    xcp.tile([P, T], bf16, tag="xc", name=f"xbf{n}_{c}")
                    for c in range(CO)
                ]
                for co in range(CO):
                    nc.gpsimd.dma_start(mbf[co], fmb[n, co * P : (co + 1) * P, :])
                    nc.scalar.dma_start(xrows[co], fxb[n, co * P : (co + 1) * P, :])
                if n == 0:
                    nc.scalar.dma_start(
                        w1t_sb[:], w1t.rearrange("(co p) j -> p co j", p=P)
                    )
                    nc.scalar.dma_start(
                        w2t_sb[:], w2t.rearrange("(jo p) i -> p jo i", p=P)
                    )
                    nc.scalar.dma_start(b1_sb[:], b1.rearrange("(jo p) -> p jo", p=P))
                    nc.scalar.dma_start(b2_sb[:], b2.rearrange("(io p) -> p io", p=P))

                def row_T(dst, rb, co):
                    nc.sync.dma_start(
                        dst[:, :, co * P : (co + 1) * P], rb, transpose=True
                    )

                # stage A consumes M^T cols 0-511 (M rows 0-3) + all of X^T
                # first, then M rows 4-7
                for co in range(4):
                    row_T(mt, mbf[co], co)
                    row_T(xt, xrows[co], co)
                for co in range(4, CO):
                    row_T(xt, xrows[co], co)
                for co in range(4, CO):
                    row_T(mt, mbf[co], co)

                # ---- stage A: S = X M^T / T, A = softmax rows ----
                a = amp.tile([P, CO, C], bf16, tag="a", name=f"a{n}")
                rs2 = statp.tile([P, ND, CO], f32, tag="rs2", name=f"rs2_{n}")
                for db in range(ND):
                    for co in range(CO):
                        ps = psp.tile(
                            [P, DB], f32, tag="mm", name=f"psA{n}_{co}_{db}"
                        )
                        for to in range(TO):
                            nc.tensor.matmul(
                                ps,
                                xt[:, to, co * P : (co + 1) * P],
                                mt[:, to, db * DB : (db + 1) * DB],
                                start=(to == 0),
                                stop=(to == TO - 1),
                            )
                        # A_raw = exp(S/T); row-sums accumulate for softmax.
                        nc.scalar.activation(
                            a[:, co, db * DB : (db + 1) * DB],
                            ps,
                            AF.Exp,
                            scale=1.0 / T,
                            accum_out=rs2[:, db, co : co + 1],
                        )
                nc.vector.tensor_add(rs2[:, 0, :], rs2[:, 0, :], rs2[:, 1, :])
                rinv = rs2[:, 1, :]
                nc.vector.reciprocal(rinv, rs2[:, 0, :])
                for co in range(CO):
                    nc.vector.tensor_scalar_mul(
                        a[:, co, :], a[:, co, :], rinv[:, co : co + 1]
                    )
                # A^T via xbar (exact: natural mapping)
                at = amp.tile([P, CO, C], bf16, tag="at", name=f"at{n}")
                for co in range(CO):
                    nc.sync.dma_start(
                        at[:, :, co * P : (co + 1) * P], a[:, co, :], transpose=True
                    )

                def cross(v, lhs_t, rows, name):
                    """v[:, o, t] = sum_k lhs_t[:, k, o-tile].T @ rows[k] + rows[o]"""
                    for tb in range(NT):
                        for o in range(CO):
                            ps = psp.tile(
                                [P, TB], f32, tag="mm", name=f"ps{name}_{tb}_{o}"
                            )
                            for k in range(CO):
                                nc.tensor.matmul(
                                    ps,
                                    lhs_t[:, k, o * P : (o + 1) * P],
                                    rows[k][:, tb * TB : (tb + 1) * TB],
                                    start=(k == 0),
                                    stop=(k == CO - 1),
                                )
                            nc.vector.tensor_add(
                                v[:, o, tb * TB : (tb + 1) * TB],
                                ps,
                                rows[o][:, tb * TB : (tb + 1) * TB],
                            )

                def ffn(v, resid_of, odst, seq, store_engs):
                    for tb in range(NT):
                        h1 = h1p.tile(
                            [P, CO, TB], bf16, tag="h1", name=f"h1_{n}_{seq}_{tb}"
                        )
                        for jo in range(CO):
                            ps = psp.tile(
                                [P, TB], f32, tag="mm", name=f"ps1_{n}_{seq}_{tb}_{jo}"
                            )
                            for co in range(CO):
                                nc.tensor.matmul(
                                    ps,
                                    w1t_sb[:, co, jo * P : (jo + 1) * P],
                                    v[:, co, tb * TB : (tb + 1) * TB],
                                    start=(co == 0),
                                    stop=(co == CO - 1),
                                )
                            nc.scalar.activation(
                                h1[:, jo, :],
                                ps,
                                AF_ACT,
                                bias=b1_sb[:, jo : jo + 1],
                                scale=1.0,
                            )
                        for io in range(CO):
                            ps = psp.tile(
                                [P, TB], f32, tag="mm", name=f"ps2_{n}_{seq}_{tb}_{io}"
                            )
                            for jo in range(CO):
                                nc.tensor.matmul(
                                    ps,
                                    w2t_sb[:, jo, io * P : (io + 1) * P],
                                    h1[:, jo, :],
                                    start=(jo == 0),
                                    stop=(jo == CO - 1),
                                )
                            st = outstp.tile(
                                [P, TB],
                                f32,
                                tag="outst",
                                name=f"st_{n}_{seq}_{tb}_{io}",
                            )
                            nc.scalar.activation(
                                st, ps, AF.Identity, bias=b2_sb[:, io : io + 1]
                            )
                            rt = resid_of(io, tb)
                            nc.vector.tensor_add(st, st, rt)
                            getattr(nc, store_engs[io % 2]).dma_start(
                                odst[
                                    n,
                                    io * P : (io + 1) * P,
                                    tb * TB : (tb + 1) * TB,
                                ],
                                st,
                            )

                # ---- cross_x = A^T X + X (X rows resident) ----
                vx = shp.tile([P, CO, T], bf16, tag="sh", name=f"vx{n}")
                cross(vx, a, xrows, f"X{n}")

                # x-sequence FFN residual straight from the live X rows
                ffn(
                    vx,
                    lambda io, tb: xrows[io][:, tb * TB : (tb + 1) * TB],
                    out_x,
                    0,
                    ("scalar", "sync"),
                )

                # ---- cross_m = A M + M (M rows resident) ----
                vm = shp.tile([P, CO, T], bf16, tag="sh", name=f"vm{n}")
                cross(vm, at, mbf, f"M{n}")

                # m-sequence FFN residual from fresh small bf16 loads so the
                # M row slots free at cross_m (keeps next batch's loads early)
                def m_resid(io, tb):
                    rt = residp.tile(
                        [P, TB], bf16, tag="res", name=f"rt_{n}_{io}_{tb}"
                    )
                    nc.sync.dma_start(
                        rt, fmb[n, io * P : (io + 1) * P, tb * TB : (tb + 1) * TB]
                    )
                    return rt

                ffn(vm, m_resid, out_m, 1, ("gpsimd", "sync"))

    nc.compile()
    return nc


def get_nc(nb=NB):
    if nb not in _CACHE:
        _CACHE[nb] = _build(nb)
    return _CACHE[nb]


def make_in_maps(feature_x, feature_m, W1, b1, W2, b2):
    """Host prep: slice per core + pre-cast features/weights to bf16."""
    fxb = np.asarray(feature_x, dtype=np.float32).astype(ml_dtypes.bfloat16)
    fmb = np.asarray(feature_m, dtype=np.float32).astype(ml_dtypes.bfloat16)
    w1t = np.ascontiguousarray(np.asarray(W1, dtype=np.float32).T).astype(
        ml_dtypes.bfloat16
    )
    w2t = np.ascontiguousarray(np.asarray(W2, dtype=np.float32).T).astype(
        ml_dtypes.bfloat16
    )
    b1 = np.ascontiguousarray(np.asarray(b1, dtype=np.float32))
    b2 = np.ascontiguousarray(np.asarray(b2, dtype=np.float32))
    in_maps = []
    for k in range(NCORES):
        in_maps.append(
            {
                "fxb": np.ascontiguousarray(fxb[k * NB : (k + 1) * NB]),
                "fmb": np.ascontiguousarray(fmb[k * NB : (k + 1) * NB]),
                "w1t": w1t,
                "w2t": w2t,
                "b1": b1,
                "b2": b2,
            }
        )
    return in_maps


def run_device(in_maps, trace=False, trace_kwargs=None):
    from concourse.bass_utils import run_bass_kernel_spmd

    nc = get_nc(NB)
    return run_bass_kernel_spmd(
        nc,
        in_maps,
        core_ids=list(range(NCORES)),
        trace=trace,
        **(trace_kwargs or {}),
    )


def kernel(feature_x, feature_m, W1, b1, W2, b2):
    in_maps = make_in_maps(feature_x, feature_m, W1, b1, W2, b2)
    res = run_device(in_maps, trace=False)
    out_x = np.concatenate([r["out_x"] for r in res.results], axis=0)
    out_m = np.concatenate([r["out_m"] for r in res.results], axis=0)
    return (out_x.astype(np.float32), out_m.astype(np.float32))


# revision 19
# speedup vs baseline: 1.2750x; 1.0017x over previous
"""Trainium2 Bass kernel for nn_CrossXMFusion (dense_transformer).

Computation per batch n (C=1024 channels, T=2048 time):
  S    = X @ M^T / T                  (attention logits, contraction over t)
  A    = softmax(S, axis=-1)
  Vx   = A^T @ X + X                  (cross_x)
  Vm   = A @ M + M                    (cross_m)
  h(V) = gelu(V^T @ W1^T + b1) @ W2^T + b2   (channel-FFN, t-parallel)
  out_x = h(Vx)^T + X ; out_m = h(Vm)^T + M

Sharding: data-parallel over batch n across 8 NeuronCores (2 batches/core),
FF weights replicated, no cross-device comms.

Design notes (v4), driven by HW traces:
 - Matmuls in bf16 with fp32 PSUM accumulation; softmax in fp32.
 - Per-NC HBM one-way bandwidth measured ~200 GB/s and DMAs within one
   engine-queue complete serially, so input features are pre-cast to bf16 on
   the host (same RNE rounding the device DMA cast would apply) — halves load
   bytes and avoids the (measured ~55 GB/s) SWDGE casting-DMA path entirely.
 - X rows load on the scalar HWDGE ring, M rows on the SWDGE ring (parallel),
   transposes + small loads on the sync HWDGE ring.
 - X^T / M^T / A^T via DMA xbar transposes (exact natural 3D mapping), PE left
   entirely to matmuls.
 - Phase order per batch: stage A -> cross_x -> FFN(x) -> cross_m -> FFN(m);
   X and M bf16 rows stay resident for transpose feed, cross rhs, and
   residuals (no reloads); slot rotation frees X rows to the next batch after
   FFN(x) and M rows after cross_m.
"""

import sys

sys.path.insert(0, "/opt/trn_rl_repo")

import numpy as np
import ml_dtypes

NCORES = 8
NFULL = 16  # full batch
NB = NFULL // NCORES  # batches per core
C, T, P = 1024, 2048, 128
CO = C // P  # 8 channel tiles
TO = T // P  # 16 time tiles
TB = 512  # matmul moving free-dim block
NT = T // TB  # 4 t-blocks
DB = 512  # stage-A d block
ND = C // DB  # 2

_CACHE = {}


def _build(nb=NB, act_name="Gelu"):
    import concourse.mybir as mybir
    import concourse.tile as tile
    from concourse import bacc

    dt = mybir.dt
    AF = mybir.ActivationFunctionType
    AF_ACT = getattr(AF, act_name)
    bf16 = dt.bfloat16
    f32 = dt.float32

    nc = bacc.Bacc("TRN2", target_bir_lowering=False, debug=False, num_devices=NCORES)

    fxb = nc.dram_tensor("fxb", [nb, C, T], bf16, kind="ExternalInput")
    fmb = nc.dram_tensor("fmb", [nb, C, T], bf16, kind="ExternalInput")
    w1t = nc.dram_tensor("w1t", [C, C], bf16, kind="ExternalInput")
    w2t = nc.dram_tensor("w2t", [C, C], bf16, kind="ExternalInput")
    b1 = nc.dram_tensor("b1", [C], f32, kind="ExternalInput")
    b2 = nc.dram_tensor("b2", [C], f32, kind="ExternalInput")
    out_x = nc.dram_tensor("out_x", [nb, C, T], f32, kind="ExternalOutput")
    out_m = nc.dram_tensor("out_m", [nb, C, T], f32, kind="ExternalOutput")

    with tile.TileContext(nc) as tc:
        with (
            tc.tile_pool(name="const", bufs=1) as constp,
            # bf16 feature rows [P, T]: transpose feed + cross rhs + residuals
            tc.tile_pool(name="xcp", bufs=2) as xcp,
            tc.tile_pool(name="shp", bufs=2) as shp,
            tc.tile_pool(name="amp", bufs=1) as amp,
            tc.tile_pool(name="h1p", bufs=1) as h1p,
            tc.tile_pool(name="outstp", bufs=2) as outstp,
            tc.tile_pool(name="residp", bufs=3) as residp,
            tc.tile_pool(name="statp", bufs=1) as statp,
            tc.tile_pool(name="psp", bufs=8, space="PSUM") as psp,
        ):
            # ---- persistent weights/biases (tiles here; DMAs issued after
            # batch 0's feature rows so they don't block the scalar ring) ----
            w1t_sb = constp.tile([P, CO, C], bf16, tag="w1", name="w1t_sb")
            w2t_sb = constp.tile([P, CO, C], bf16, tag="w2", name="w2t_sb")
            b1_sb = constp.tile([P, CO], f32, tag="b1", name="b1_sb")
            b2_sb = constp.tile([P, CO], f32, tag="b2", name="b2_sb")

            for n in range(nb):
                # ---- bf16 feature loads (2-row chunks, X on scalar ring, M
                # on SWDGE ring in parallel) + batched xbar transposes.
                # Transposed layout xt2/mt2[p, c-block, t-tile, q] =
                # X[cb*128+q, tt*128+p]; 2-row transpose chunks cost one
                # sequencer issue each instead of eight.
                mt2 = shp.tile([P, CO, TO, P], bf16, tag="sh", name=f"mt{n}")
                xt2 = shp.tile([P, CO, TO, P], bf16, tag="sh", name=f"xt{n}")
                mbf = xcp.tile([P, CO, T], bf16, tag="xc", name=f"mbf{n}")
                xrow = xcp.tile([P, CO, T], bf16, tag="xc", name=f"xbf{n}")
                for k in range(4):
                    nc.gpsimd.dma_start(
                        mbf[:, 2 * k : 2 * k + 2, :],
                        fmb[n, k * 2 * P : (k + 1) * 2 * P, :].rearrange(
                            "(c p) t -> p c t", p=P
                        ),
                    )
                    nc.scalar.dma_start(
                        xrow[:, 2 * k : 2 * k + 2, :],
                        fxb[n, k * 2 * P : (k + 1) * 2 * P, :].rearrange(
                            "(c p) t -> p c t", p=P
                        ),
                    )
                if n == 0:
                    nc.scalar.dma_start(
                        w1t_sb[:], w1t.rearrange("(co p) j -> p co j", p=P)
                    )
                    nc.scalar.dma_start(
                        w2t_sb[:], w2t.rearrange("(jo p) i -> p jo i", p=P)
                    )
                    nc.scalar.dma_start(b1_sb[:], b1.rearrange("(jo p) -> p jo", p=P))
                    nc.scalar.dma_start(b2_sb[:], b2.rearrange("(io p) -> p io", p=P))
                for k in range(4):
                    nc.sync.dma_start(
                        mt2[:, 2 * k : 2 * k + 2, :, :],
                        mbf[:, 2 * k : 2 * k + 2, :],
                        transpose=True,
                    )
                    nc.scalar.dma_start(
                        xt2[:, 2 * k : 2 * k + 2, :, :],
                        xrow[:, 2 * k : 2 * k + 2, :],
                        transpose=True,
                    )

                # ---- stage A: S = X M^T / T, A = softmax rows ----
                a = amp.tile([P, CO, C], bf16, tag="a", name=f"a{n}")
                rs2 = statp.tile([P, ND, CO], f32, tag="rs2", name=f"rs2_{n}")
                for db in range(ND):
                    for co in range(CO):
                        ps = psp.tile(
                            [P, DB], f32, tag="mm", name=f"psA{n}_{co}_{db}"
                        )
                        for to in range(TO):
                            nc.tensor.matmul(
                                ps,
                                xt2[:, co, to, :],
                                mt2[:, db * 4 : (db + 1) * 4, to, :],
                                start=(to == 0),
                                stop=(to == TO - 1),
                            )
                        # A_raw = exp(S/T); row-sums accumulate for softmax.
                        nc.scalar.activation(
                            a[:, co, db * DB : (db + 1) * DB],
                            ps,
                            AF.Exp,
                            scale=1.0 / T,
                            accum_out=rs2[:, db, co : co + 1],
                        )
                nc.vector.tensor_add(rs2[:, 0, :], rs2[:, 0, :], rs2[:, 1, :])
                rinv = rs2[:, 1, :]
                nc.vector.reciprocal(rinv, rs2[:, 0, :])
                for co in range(CO):
                    nc.vector.tensor_scalar_mul(
                        a[:, co, :], a[:, co, :], rinv[:, co : co + 1]
                    )
                # A^T via one batched xbar transpose:
                # at2[p, cb, do, q] = A[cb*128+q, do*128+p]
                at2 = amp.tile([P, CO, CO, P], bf16, tag="at", name=f"at{n}")
                nc.sync.dma_start(at2[:], a[:], transpose=True)

                def cross(v, lhs_of, rows, name):
                    """v[:, o, t] = sum_k lhs(k, o).T @ rows[:, k, t] + rows[:, o, t]"""
                    for tb in range(NT):
                        for o in range(CO):
                            ps = psp.tile(
                                [P, TB], f32, tag="mm", name=f"ps{name}_{tb}_{o}"
                            )
                            for k in range(CO):
                                nc.tensor.matmul(
                                    ps,
                                    lhs_of(k, o),
                                    rows[:, k, tb * TB : (tb + 1) * TB],
                                    start=(k == 0),
                                    stop=(k == CO - 1),
                                )
                            nc.vector.tensor_add(
                                v[:, o, tb * TB : (tb + 1) * TB],
                                ps,
                                rows[:, o, tb * TB : (tb + 1) * TB],
                            )

                def ffn(v, resid_of, odst, seq, store_engs):
                    for tb in range(NT):
                        h1 = h1p.tile(
                            [P, CO, TB], bf16, tag="h1", name=f"h1_{n}_{seq}_{tb}"
                        )
                        for jo in range(CO):
                            ps = psp.tile(
                                [P, TB], f32, tag="mm", name=f"ps1_{n}_{seq}_{tb}_{jo}"
                            )
                            for co in range(CO):
                                nc.tensor.matmul(
                                    ps,
                                    w1t_sb[:, co, jo * P : (jo + 1) * P],
                                    v[:, co, tb * TB : (tb + 1) * TB],
                                    start=(co == 0),
                                    stop=(co == CO - 1),
                                )
                            nc.scalar.activation(
                                h1[:, jo, :],
                                ps,
                                AF_ACT,
                                bias=b1_sb[:, jo : jo + 1],
                                scale=1.0,
                            )
                        for io in range(CO):
                            ps = psp.tile(
                                [P, TB], f32, tag="mm", name=f"ps2_{n}_{seq}_{tb}_{io}"
                            )
                            for jo in range(CO):
                                nc.tensor.matmul(
                                    ps,
                                    w2t_sb[:, jo, io * P : (io + 1) * P],
                                    h1[:, jo, :],
                                    start=(jo == 0),
                                    stop=(jo == CO - 1),
                                )
                            st = outstp.tile(
                                [P, TB],
                                f32,
                                tag="outst",
                                name=f"st_{n}_{seq}_{tb}_{io}",
                            )
                            nc.scalar.activation(
                                st, ps, AF.Identity, bias=b2_sb[:, io : io + 1]
                            )
                            rt = resid_of(io, tb)
                            nc.vector.tensor_add(st, st, rt)
                            getattr(nc, store_engs[io % 2]).dma_start(
                                odst[
                                    n,
                                    io * P : (io + 1) * P,
                                    tb * TB : (tb + 1) * TB,
                                ],
                                st,
                            )

                # ---- cross_x = A^T X + X (X rows resident) ----
                vx = shp.tile([P, CO, T], bf16, tag="sh", name=f"vx{n}")
                cross(vx, lambda k, o: a[:, k, o * P : (o + 1) * P], xrow, f"X{n}")

                # x-sequence FFN residual straight from the live X rows
                ffn(
                    vx,
                    lambda io, tb: xrow[:, io, tb * TB : (tb + 1) * TB],
                    out_x,
                    0,
                    ("scalar", "sync"),
                )

                # ---- cross_m = A M + M (M rows resident) ----
                vm = shp.tile([P, CO, T], bf16, tag="sh", name=f"vm{n}")
                cross(vm, lambda k, o: at2[:, o, k, :], mbf, f"M{n}")

                # m-sequence FFN residual from fresh small bf16 loads so the
                # M row slots free at cross_m (keeps next batch's loads early)
                def m_resid(io, tb):
                    rt = residp.tile(
                        [P, TB], bf16, tag="res", name=f"rt_{n}_{io}_{tb}"
                    )
                    nc.scalar.dma_start(
                        rt, fmb[n, io * P : (io + 1) * P, tb * TB : (tb + 1) * TB]
                    )
                    return rt

                ffn(vm, m_resid, out_m, 1, ("gpsimd", "sync"))

    nc.compile()
    return nc


def get_nc(nb=NB):
    if nb not in _CACHE:
        _CACHE[nb] = _build(nb)
    return _CACHE[nb]


def make_in_maps(feature_x, feature_m, W1, b1, W2, b2):
    """Host prep: slice per core + pre-cast features/weights to bf16."""
    fxb = np.asarray(feature_x, dtype=np.float32).astype(ml_dtypes.bfloat16)
    fmb = np.asarray(feature_m, dtype=np.float32).astype(ml_dtypes.bfloat16)
    w1t = np.ascontiguousarray(np.asarray(W1, dtype=np.float32).T).astype(
        ml_dtypes.bfloat16
    )
    w2t = np.ascontiguousarray(np.asarray(W2, dtype=np.float32).T).astype(
        ml_dtypes.bfloat16
    )
    b1 = np.ascontiguousarray(np.asarray(b1, dtype=np.float32))
    b2 = np.ascontiguousarray(np.asarray(b2, dtype=np.float32))
    in_maps = []
    for k in range(NCORES):
        in_maps.append(
            {
                "fxb": np.ascontiguousarray(fxb[k * NB : (k + 1) * NB]),
                "fmb": np.ascontiguousarray(fmb[k * NB : (k + 1) * NB]),
                "w1t": w1t,
                "w2t": w2t,
                "b1": b1,
                "b2": b2,
            }
        )
    return in_maps


def run_device(in_maps, trace=False, trace_kwargs=None):
    from concourse.bass_utils import run_bass_kernel_spmd

    nc = get_nc(NB)
    return run_bass_kernel_spmd(
        nc,
        in_maps,
        core_ids=list(range(NCORES)),
        trace=trace,
        **(trace_kwargs or {}),
    )


def kernel(feature_x, feature_m, W1, b1, W2, b2):
    in_maps = make_in_maps(feature_x, feature_m, W1, b1, W2, b2)
    res = run_device(in_maps, trace=False)
    out_x = np.concatenate([r["out_x"] for r in res.results], axis=0)
    out_m = np.concatenate([r["out_m"] for r in res.results], axis=0)
    return (out_x.astype(np.float32), out_m.astype(np.float32))


# revision 21
# speedup vs baseline: 1.2918x; 1.0131x over previous
"""Trainium2 Bass kernel for nn_CrossXMFusion (dense_transformer).

Computation per batch n (C=1024 channels, T=2048 time):
  S    = X @ M^T / T                  (attention logits, contraction over t)
  A    = softmax(S, axis=-1)
  Vx   = A^T @ X + X                  (cross_x)
  Vm   = A @ M + M                    (cross_m)
  h(V) = gelu(V^T @ W1^T + b1) @ W2^T + b2   (channel-FFN, t-parallel)
  out_x = h(Vx)^T + X ; out_m = h(Vm)^T + M

Sharding: data-parallel over batch n across 8 NeuronCores (2 batches/core),
FF weights replicated, no cross-device comms.

Design notes (v4), driven by HW traces:
 - Matmuls in bf16 with fp32 PSUM accumulation; softmax in fp32.
 - Per-NC HBM one-way bandwidth measured ~200 GB/s and DMAs within one
   engine-queue complete serially, so input features are pre-cast to bf16 on
   the host (same RNE rounding the device DMA cast would apply) — halves load
   bytes and avoids the (measured ~55 GB/s) SWDGE casting-DMA path entirely.
 - X rows load on the scalar HWDGE ring, M rows on the SWDGE ring (parallel),
   transposes + small loads on the sync HWDGE ring.
 - X^T / M^T / A^T via DMA xbar transposes (exact natural 3D mapping), PE left
   entirely to matmuls.
 - Phase order per batch: stage A -> cross_x -> FFN(x) -> cross_m -> FFN(m);
   X and M bf16 rows stay resident for transpose feed, cross rhs, and
   residuals (no reloads); slot rotation frees X rows to the next batch after
   FFN(x) and M rows after cross_m.
"""

import sys

sys.path.insert(0, "/opt/trn_rl_repo")

import numpy as np
import ml_dtypes

NCORES = 8
NFULL = 16  # full batch
NB = NFULL // NCORES  # batches per core
C, T, P = 1024, 2048, 128
CO = C // P  # 8 channel tiles
TO = T // P  # 16 time tiles
TB = 512  # matmul moving free-dim block
NT = T // TB  # 4 t-blocks
DB = 512  # stage-A d block
ND = C // DB  # 2

_CACHE = {}


def _build(nb=NB, act_name="Gelu"):
    import concourse.mybir as mybir
    import concourse.tile as tile
    from concourse import bacc

    dt = mybir.dt
    AF = mybir.ActivationFunctionType
    AF_ACT = getattr(AF, act_name)
    bf16 = dt.bfloat16
    f32 = dt.float32

    nc = bacc.Bacc("TRN2", target_bir_lowering=False, debug=False, num_devices=NCORES)

    fxb = nc.dram_tensor("fxb", [nb, C, T], bf16, kind="ExternalInput")
    fmb = nc.dram_tensor("fmb", [nb, C, T], bf16, kind="ExternalInput")
    w1t = nc.dram_tensor("w1t", [C, C], bf16, kind="ExternalInput")
    w2t = nc.dram_tensor("w2t", [C, C], bf16, kind="ExternalInput")
    b1 = nc.dram_tensor("b1", [C], f32, kind="ExternalInput")
    b2 = nc.dram_tensor("b2", [C], f32, kind="ExternalInput")
    out_x = nc.dram_tensor("out_x", [nb, C, T], f32, kind="ExternalOutput")
    out_m = nc.dram_tensor("out_m", [nb, C, T], f32, kind="ExternalOutput")

    with tile.TileContext(nc) as tc:
        with (
            tc.tile_pool(name="const", bufs=1) as constp,
            # bf16 feature rows [P, T]: transpose feed + cross rhs + residuals
            tc.tile_pool(name="xcp", bufs=2) as xcp,
            tc.tile_pool(name="shp", bufs=2) as shp,
            tc.tile_pool(name="amp", bufs=1) as amp,
            tc.tile_pool(name="h1p", bufs=1) as h1p,
            tc.tile_pool(name="outstp", bufs=2) as outstp,
            tc.tile_pool(name="residp", bufs=3) as residp,
            tc.tile_pool(name="statp", bufs=1) as statp,
            tc.tile_pool(name="psp", bufs=8, space="PSUM") as psp,
        ):
            # ---- persistent weights/biases (tiles here; DMAs issued after
            # batch 0's feature rows so they don't block the scalar ring) ----
            w1t_sb = constp.tile([P, CO, C], bf16, tag="w1", name="w1t_sb")
            w2t_sb = constp.tile([P, CO, C], bf16, tag="w2", name="w2t_sb")
            b1_sb = constp.tile([P, CO], f32, tag="b1", name="b1_sb")
            b2_sb = constp.tile([P, CO], f32, tag="b2", name="b2_sb")

            for n in range(nb):
                # ---- bf16 feature loads (2-row chunks, X on scalar ring, M
                # on SWDGE ring in parallel) + batched xbar transposes.
                # Transposed layout xt2/mt2[p, c-block, t-tile, q] =
                # X[cb*128+q, tt*128+p]; 2-row transpose chunks cost one
                # sequencer issue each instead of eight.
                mt2 = shp.tile([P, CO, TO, P], bf16, tag="sh", name=f"mt{n}")
                xt2 = shp.tile([P, CO, TO, P], bf16, tag="sh", name=f"xt{n}")
                mbf = xcp.tile([P, CO, T], bf16, tag="xc", name=f"mbf{n}")
                xrow = xcp.tile([P, CO, T], bf16, tag="xc", name=f"xbf{n}")
                for k in range(4):
                    nc.gpsimd.dma_start(
                        mbf[:, 2 * k : 2 * k + 2, :],
                        fmb[n, k * 2 * P : (k + 1) * 2 * P, :].rearrange(
                            "(c p) t -> p c t", p=P
                        ),
                    )
                    nc.scalar.dma_start(
                        xrow[:, 2 * k : 2 * k + 2, :],
                        fxb[n, k * 2 * P : (k + 1) * 2 * P, :].rearrange(
                            "(c p) t -> p c t", p=P
                        ),
                    )
                if n == 0:
                    nc.gpsimd.dma_start(
                        w1t_sb[:], w1t.rearrange("(co p) j -> p co j", p=P)
                    )
                    nc.gpsimd.dma_start(
                        w2t_sb[:], w2t.rearrange("(jo p) i -> p jo i", p=P)
                    )
                    nc.gpsimd.dma_start(b1_sb[:], b1.rearrange("(jo p) -> p jo", p=P))
                    nc.gpsimd.dma_start(b2_sb[:], b2.rearrange("(io p) -> p io", p=P))
                for k in range(4):
                    nc.sync.dma_start(
                        mt2[:, 2 * k : 2 * k + 2, :, :],
                        mbf[:, 2 * k : 2 * k + 2, :],
                        transpose=True,
                    )
                    nc.scalar.dma_start(
                        xt2[:, 2 * k : 2 * k + 2, :, :],
                        xrow[:, 2 * k : 2 * k + 2, :],
                        transpose=True,
                    )

                # ---- stage A: S = X M^T / T, A = softmax rows ----
                a = amp.tile([P, CO, C], bf16, tag="a", name=f"a{n}")
                rs2 = statp.tile([P, ND, CO], f32, tag="rs2", name=f"rs2_{n}")
                for db in range(ND):
                    for co in range(CO):
                        ps = psp.tile(
                            [P, DB], f32, tag="mm", name=f"psA{n}_{co}_{db}"
                        )
                        for to in range(TO):
                            nc.tensor.matmul(
                                ps,
                                xt2[:, co, to, :],
                                mt2[:, db * 4 : (db + 1) * 4, to, :],
                                start=(to == 0),
                                stop=(to == TO - 1),
                            )
                        # A_raw = exp(S/T); row-sums accumulate for softmax.
                        nc.scalar.activation(
                            a[:, co, db * DB : (db + 1) * DB],
                            ps,
                            AF.Exp,
                            scale=1.0 / T,
                            accum_out=rs2[:, db, co : co + 1],
                        )
                nc.vector.tensor_add(rs2[:, 0, :], rs2[:, 0, :], rs2[:, 1, :])
                rinv = rs2[:, 1, :]
                nc.vector.reciprocal(rinv, rs2[:, 0, :])
                for co in range(CO):
                    nc.vector.tensor_scalar_mul(
                        a[:, co, :], a[:, co, :], rinv[:, co : co + 1]
                    )
                # A^T via one batched xbar transpose:
                # at2[p, cb, do, q] = A[cb*128+q, do*128+p]
                at2 = amp.tile([P, CO, CO, P], bf16, tag="at", name=f"at{n}")
                nc.sync.dma_start(at2[:], a[:], transpose=True)

                def cross(v, lhs_of, rows, name):
                    """v[:, o, t] = sum_k lhs(k, o).T @ rows[:, k, t] + rows[:, o, t]"""
                    for tb in range(NT):
                        sl = slice(tb * TB, (tb + 1) * TB)
                        for o in range(CO):
                            ps = psp.tile(
                                [P, TB], f32, tag="mm", name=f"ps{name}_{tb}_{o}"
                            )
                            for k in range(CO):
                                nc.tensor.matmul(
                                    ps,
                                    lhs_of(k, o),
                                    rows[:, k, sl],
                                    start=(k == 0),
                                    stop=(k == CO - 1),
                                )
                            nc.vector.tensor_add(
                                v[:, o, sl], ps, rows[:, o, sl]
                            )

                def ffn(v, resid_of, odst, seq, store_engs):
                    for tb in range(NT):
                        h1 = h1p.tile(
                            [P, CO, TB], bf16, tag="h1", name=f"h1_{n}_{seq}_{tb}"
                        )
                        for jo in range(CO):
                            ps = psp.tile(
                                [P, TB], f32, tag="mm", name=f"ps1_{n}_{seq}_{tb}_{jo}"
                            )
                            for co in range(CO):
                                nc.tensor.matmul(
                                    ps,
                                    w1t_sb[:, co, jo * P : (jo + 1) * P],
                                    v[:, co, tb * TB : (tb + 1) * TB],
                                    start=(co == 0),
                                    stop=(co == CO - 1),
                                )
                            nc.scalar.activation(
                                h1[:, jo, :],
                                ps,
                                AF_ACT,
                                bias=b1_sb[:, jo : jo + 1],
                                scale=1.0,
                            )
                        for io in range(CO):
                            ps = psp.tile(
                                [P, TB], f32, tag="mm", name=f"ps2_{n}_{seq}_{tb}_{io}"
                            )
                            for jo in range(CO):
                                nc.tensor.matmul(
                                    ps,
                                    w2t_sb[:, jo, io * P : (io + 1) * P],
                                    h1[:, jo, :],
                                    start=(jo == 0),
                                    stop=(jo == CO - 1),
                                )
                            st = outstp.tile(
                                [P, TB],
                                f32,
                                tag="outst",
                                name=f"st_{n}_{seq}_{tb}_{io}",
                            )
                            nc.scalar.activation(
                                st, ps, AF.Identity, bias=b2_sb[:, io : io + 1]
                            )
                            rt = resid_of(io, tb)
                            nc.vector.tensor_add(st, st, rt)
                            getattr(nc, store_engs[io % 2]).dma_start(
                                odst[
                                    n,
                                    io * P : (io + 1) * P,
                                    tb * TB : (tb + 1) * TB,
                                ],
                                st,
                            )

                # ---- cross_x = A^T X + X (X rows resident) ----
                vx = shp.tile([P, CO, T], bf16, tag="sh", name=f"vx{n}")
                cross(vx, lambda k, o: a[:, k, o * P : (o + 1) * P], xrow, f"X{n}")

                # x-sequence FFN residual straight from the live X rows
                ffn(
                    vx,
                    lambda io, tb: xrow[:, io, tb * TB : (tb + 1) * TB],
                    out_x,
                    0,
                    ("scalar", "sync"),
                )

                # ---- cross_m = A M + M (M rows resident) ----
                vm = shp.tile([P, CO, T], bf16, tag="sh", name=f"vm{n}")
                cross(vm, lambda k, o: at2[:, o, k, :], mbf, f"M{n}")

                # m-sequence FFN residual from fresh small bf16 loads so the
                # M row slots free at cross_m (keeps next batch's loads early)
                def m_resid(io, tb):
                    rt = residp.tile(
                        [P, TB], bf16, tag="res", name=f"rt_{n}_{io}_{tb}"
                    )
                    nc.scalar.dma_start(
                        rt, fmb[n, io * P : (io + 1) * P, tb * TB : (tb + 1) * TB]
                    )
                    return rt

                ffn(vm, m_resid, out_m, 1, ("gpsimd", "sync"))

    nc.compile()
    return nc


def get_nc(nb=NB):
    if nb not in _CACHE:
        _CACHE[nb] = _build(nb)
    return _CACHE[nb]


def make_in_maps(feature_x, feature_m, W1, b1, W2, b2):
    """Host prep: slice per core + pre-cast features/weights to bf16."""
    fxb = np.asarray(feature_x, dtype=np.float32).astype(ml_dtypes.bfloat16)
    fmb = np.asarray(feature_m, dtype=np.float32).astype(ml_dtypes.bfloat16)
    w1t = np.ascontiguousarray(np.asarray(W1, dtype=np.float32).T).astype(
        ml_dtypes.bfloat16
    )
    w2t = np.ascontiguousarray(np.asarray(W2, dtype=np.float32).T).astype(
        ml_dtypes.bfloat16
    )
    b1 = np.ascontiguousarray(np.asarray(b1, dtype=np.float32))
    b2 = np.ascontiguousarray(np.asarray(b2, dtype=np.float32))
    in_maps = []
    for k in range(NCORES):
        in_maps.append(
            {
                "fxb": np.ascontiguousarray(fxb[k * NB : (k + 1) * NB]),
                "fmb": np.ascontiguousarray(fmb[k * NB : (k + 1) * NB]),
                "w1t": w1t,
                "w2t": w2t,
                "b1": b1,
                "b2": b2,
            }
        )
    return in_maps


def run_device(in_maps, trace=False, trace_kwargs=None):
    from concourse.bass_utils import run_bass_kernel_spmd

    nc = get_nc(NB)
    return run_bass_kernel_spmd(
        nc,
        in_maps,
        core_ids=list(range(NCORES)),
        trace=trace,
        **(trace_kwargs or {}),
    )


def kernel(feature_x, feature_m, W1, b1, W2, b2):
    in_maps = make_in_maps(feature_x, feature_m, W1, b1, W2, b2)
    res = run_device(in_maps, trace=False)
    out_x = np.concatenate([r["out_x"] for r in res.results], axis=0)
    out_m = np.concatenate([r["out_m"] for r in res.results], axis=0)
    return (out_x.astype(np.float32), out_m.astype(np.float32))
